# revision 10
# baseline (speedup 1.0000x reference)
"""CapsuleLayer dynamic-routing kernel for 8 Trainium2 NeuronCores — v4.1.

I-sharding: each core owns 144 of the 1152 input capsules.

s0 = (1/N) sum_i hat_i over ALL 1152 i is computed LOCALLY on every core
from bf16 copies of the full X/W via a dense (i,k)-packed matmul chain
(72 K=128 steps) — no AllReduce #1.  A dummy collective (alone on the
gpsimd queue) at t=0 absorbs the ~50us first-collective bootstrap so
AllReduce #2 runs lean.

Create (PE, bf16): per-i hat matmuls 4x row-tiled (tile_position=(32q,0)),
4 i's -> one [128,2048] 4-bank PSUM tile; evac fp32->bf16 on ScalarE only,
so the DVE round-1 proj pass pipelines underneath.  hat is SBUF bf16
[b, i, (d,n)], n contiguous.

Routing (DVE, BI=16 blocks, hot TT ops in 2x bf16 mode):
  proj: tmp = hat_blk * rep4; log-tree over d -> bb[b,i,n]; per-block exp
  softmax: DVE tree over n + reciprocal
  wsum: tmp = hat_blk * c (bcast over d); log-tree over i -> sblk; cross tree
Round 2 ships per-core partial s2; host sums partials + squashes in numpy.
"""

import os
import numpy as np
import ml_dtypes

import concourse.bass as bass
import concourse.bacc as bacc
import concourse.tile as tile
import concourse.mybir as mybir
from concourse import bass_utils

B, I, DIN = 128, 1152, 8
N, D = 32, 16
ND = N * D  # 512
NCORES = 8
IL = I // NCORES          # 144
G = IL // 4               # 36 groups of 4 i's
WCH = 6                   # create stream chunk: 6 groups
NCH = G // WCH            # 6 waves
KF = I * DIN // 128       # 72 dense-packed K chunks for s0 (full I)
WV = 8                    # s0 W stream: 8 chunks per wave
NWV = KF // WV            # 9 waves
BI = 16                   # i-block for routing passes
NBLK = IL // BI           # 9
EPS = 1e-7
ROUTINGS = 3
F32 = mybir.dt.float32
BF16 = mybir.dt.bfloat16
BF = ml_dtypes.bfloat16


def _ap(ap: bass.AP, dims, extra_off=0) -> bass.AP:
    return bass.AP(tensor=ap.tensor, offset=ap.offset + extra_off,
                   ap=[ap.ap[0]] + list(dims))


def build_nc():
    nc = bacc.Bacc(
        "TRN2",
        target_bir_lowering=False,
        debug=False,
        enable_asserts=True,
        num_devices=NCORES,
    )
    xq_d = nc.dram_tensor("xq", [4, DIN, G, B], BF16, kind="ExternalInput").ap()
    wq_d = nc.dram_tensor("wq", [4, DIN, G, ND], BF16, kind="ExternalInput").ap()
    xdf_d = nc.dram_tensor("xdf", [128, KF, B], BF16, kind="ExternalInput").ap()
    wdf_d = nc.dram_tensor("wdf", [128, KF, ND], BF16, kind="ExternalInput").ap()
    out_d = nc.dram_tensor("out", [B, ND], F32, kind="ExternalOutput").ap()

    with tile.TileContext(nc) as tc:
        with (
            tc.tile_pool(name="big", bufs=1) as big,
            tc.tile_pool(name="ps", bufs=2, space="PSUM") as pspool,
            tc.tile_pool(name="dram", bufs=1, space="DRAM") as dram,
        ):
            hat = big.tile([B, IL, ND], BF16)           # 144 KB/part
            scratch = big.tile([B, 19456], BF16)        # 38 KB/part union
            bb = big.tile([B, IL, N], BF16)             # 9 KB
            ee = big.tile([B, IL, N], BF16)             # 9 KB
            big3 = big.tile([B, 3, ND], F32)            # 6 KB
            s_sb, outv, tsq = (big3[:, j, :] for j in range(3))
            smalls = big.tile([B, 8, N], F32)           # 1 KB
            s2, a1, r1, rt = (smalls[:, j, :] for j in range(4))
            eps_t = smalls[:, 4, 0:1]
            dum_src = smalls[:, 6, 0:8]
            dum_dst = smalls[:, 7, 0:8]
            den = big.tile([B, IL], F32)                # 0.6 KB

            def sv(off, dims):
                return _ap(scratch, dims, extra_off=off)

            # -- elem map (bf16) --
            # routing: tmp 0..8192 | rep4 8192..10240 | sblk 10240..14848
            # create : xq waves 14848..16384 | wq wave 16384..19456
            # s0     : xdf 0..9216 | wdf waves 9216..13312, 13312..17408
            tmp = sv(0, [[ND, BI], [1, ND]])
            tmp4 = sv(0, [[ND, BI], [N, D], [1, N]])
            rep4 = sv(8192, [[ND, 4], [1, ND]])
            sblk = sv(10240, [[ND, NBLK], [1, ND]])
            xqw = [sv(14848 + j * 768, [[B, WCH], [1, B]]) for j in range(2)]
            wqw = sv(16384, [[ND, WCH], [1, ND]])
            xdf_sb = sv(0, [[B, KF], [1, B]])
            wdfw = [sv(9216 + j * WV * ND, [[ND, WV], [1, ND]]) for j in range(2)]

            nc.vector.memset(eps_t, EPS)
            nc.vector.memset(dum_src, 0.0)

            # ---------- dummy collective: absorb CC bootstrap ----------
            dum_in = dram.tile([B, 8], F32, tag="dumin")
            dum_out = dram.tile([B, 8], F32, tag="dumout")
            nc.gpsimd.dma_start(out=dum_in[:], in_=dum_src)
            nc.gpsimd.collective_compute(
                "AllReduce", mybir.AluOpType.add,
                replica_groups=[list(range(NCORES))],
                ins=[dum_in.opt()], outs=[dum_out.opt()])
            nc.gpsimd.dma_start(out=dum_dst, in_=dum_out[:])

            # ---------- s0 over ALL i (bf16 dense chain, no collective) ---
            # xdf + alternate wdf waves split over sync/tensor DMA queues
            nc.sync.dma_start(out=xdf_sb, in_=xdf_d[:, :, :])
            s0ps = pspool.tile([B, ND], F32, tag="ps")
            for wv in range(NWV):
                wt = wdfw[wv % 2]
                eng = nc.sync if wv % 2 == 0 else nc.scalar
                eng.dma_start(
                    out=wt, in_=wdf_d[:, wv * WV:(wv + 1) * WV, :])
                for j in range(WV):
                    c = wv * WV + j
                    nc.tensor.matmul(
                        s0ps[:], lhsT=xdf_sb[:, c, :], rhs=wt[:, j, :],
                        start=(c == 0), stop=(c == KF - 1),
                    )
            nc.scalar.mul(out=s_sb, in_=s0ps[:], mul=1.0 / N)

            s_bf = sblk[:, 1, :]  # bf16 AllReduce result staging

            def allreduce_s():
                # bf16 payload halves the ring traffic; input is the
                # already-bf16 cross-tree result in sblk[:,0,:]
                ar_in = dram.tile([B, ND], BF16, tag="arin")
                ar_out = dram.tile([B, ND], BF16, tag="arout")
                nc.gpsimd.dma_start(out=ar_in[:], in_=sblk[:, 0, :])
                nc.gpsimd.collective_compute(
                    "AllReduce", mybir.AluOpType.add,
                    replica_groups=[list(range(NCORES))],
                    ins=[ar_in.opt()], outs=[ar_out.opt()])
                nc.gpsimd.dma_start(out=s_bf, in_=ar_out[:])

            # ---------- create hat (evac on ScalarE only) ----------
            for ch in range(NCH):
                x_t = xqw[ch % 2]
                for q in range(4):
                    nc.gpsimd.dma_start(
                        out=x_t[32 * q:32 * q + DIN, :, :],
                        in_=xq_d[q, :, ch * WCH:(ch + 1) * WCH, :])
                    nc.gpsimd.dma_start(
                        out=wqw[32 * q:32 * q + DIN, :, :],
                        in_=wq_d[q, :, ch * WCH:(ch + 1) * WCH, :])
                for j in range(WCH):
                    g = ch * WCH + j
                    ps = pspool.tile([B, 4 * ND], F32, tag="ps")
                    for q in range(4):
                        nc.tensor.matmul(
                            ps[:, q * ND:(q + 1) * ND],
                            lhsT=x_t[32 * q:32 * q + DIN, j, :],
                            rhs=wqw[32 * q:32 * q + DIN, j, :],
                            start=True, stop=True,
                            tile_position=(32 * q, 0),
                        )
                    nc.scalar.copy(
                        out=_ap(hat[:, 4 * g, :], [[1, 4 * ND]]), in_=ps[:])

            def squash(src):
                nc.vector.tensor_mul(tsq, src, src)
                nc.vector.tensor_add(
                    _ap(tsq, [[N, 8], [1, N]]),
                    _ap(tsq, [[N, 8], [1, N]]),
                    _ap(tsq, [[N, 8], [1, N]], extra_off=8 * N))
                nc.vector.tensor_add(
                    _ap(tsq, [[N, 4], [1, N]]),
                    _ap(tsq, [[N, 4], [1, N]]),
                    _ap(tsq, [[N, 4], [1, N]], extra_off=4 * N))
                nc.vector.tensor_add(
                    _ap(tsq, [[N, 2], [1, N]]),
                    _ap(tsq, [[N, 2], [1, N]]),
                    _ap(tsq, [[N, 2], [1, N]], extra_off=2 * N))
                nc.vector.tensor_add(
                    s2, _ap(tsq, [[1, N]]), _ap(tsq, [[1, N]], extra_off=N))
                nc.scalar.add(out=a1, in_=s2, add=1.0)
                nc.vector.reciprocal(out=r1, in_=a1)
                nc.vector.tensor_mul(r1, r1, s2)
                nc.scalar.activation(
                    out=rt, in_=s2,
                    func=mybir.ActivationFunctionType.Sqrt,
                    bias=eps_t, scale=1.0)
                nc.vector.reciprocal(out=rt, in_=rt)
                nc.vector.tensor_mul(r1, r1, rt)
                nc.vector.tensor_mul(
                    _ap(outv, [[N, D], [1, N]]),
                    _ap(src, [[N, D], [1, N]]),
                    _ap(r1, [[0, D], [1, N]]))

            def fill_rep4():
                nc.vector.tensor_copy(rep4[:, 0, :], outv)
                nc.vector.tensor_copy(
                    _ap(rep4, [[1, ND]], extra_off=ND), _ap(rep4, [[1, ND]]))
                nc.vector.tensor_copy(
                    _ap(rep4, [[1, 2 * ND]], extra_off=2 * ND),
                    _ap(rep4, [[1, 2 * ND]]))

            # ---------- routing ----------
            squash(s_sb)

            for r in range(1, ROUTINGS):
                fill_rep4()
                for blk in range(NBLK):
                    i0 = blk * BI
                    nc.vector.tensor_mul(
                        tmp[:, :, :],
                        _ap(hat[:, i0, :], [[1, BI * ND]]),
                        _ap(rep4, [[0, 4], [1, 4 * ND]]))
                    nc.vector.tensor_add(
                        _ap(tmp4, [[ND, BI], [N, 8], [1, N]]),
                        _ap(tmp4, [[ND, BI], [N, 8], [1, N]]),
                        _ap(tmp4, [[ND, BI], [N, 8], [1, N]], extra_off=8 * N))
                    nc.vector.tensor_add(
                        _ap(tmp4, [[ND, BI], [N, 4], [1, N]]),
                        _ap(tmp4, [[ND, BI], [N, 4], [1, N]]),
                        _ap(tmp4, [[ND, BI], [N, 4], [1, N]], extra_off=4 * N))
                    nc.vector.tensor_add(
                        _ap(tmp4, [[ND, BI], [N, 2], [1, N]]),
                        _ap(tmp4, [[ND, BI], [N, 2], [1, N]]),
                        _ap(tmp4, [[ND, BI], [N, 2], [1, N]], extra_off=2 * N))
                    bb_blk = _ap(bb[:, i0, :], [[N, BI], [1, N]])
                    if r == 1:
                        nc.vector.tensor_add(
                            bb_blk,
                            _ap(tmp4, [[ND, BI], [1, N]]),
                            _ap(tmp4, [[ND, BI], [1, N]], extra_off=N))
                    else:
                        nc.vector.tensor_add(
                            _ap(tmp4, [[ND, BI], [1, N]]),
                            _ap(tmp4, [[ND, BI], [1, N]]),
                            _ap(tmp4, [[ND, BI], [1, N]], extra_off=N))
                        nc.vector.tensor_add(
                            bb_blk, bb_blk, _ap(tmp4, [[ND, BI], [1, N]]))
                    nc.scalar.activation(
                        out=_ap(ee[:, i0, :], [[1, BI * N]]),
                        in_=_ap(bb[:, i0, :], [[1, BI * N]]),
                        func=mybir.ActivationFunctionType.Exp,
                        bias=eps_t, scale=1.0)
                # ---- softmax denominator ----
                nc.vector.tensor_add(
                    _ap(tmp, [[16, IL], [1, 16]]),
                    _ap(ee[:, 0, :], [[N, IL], [1, 16]]),
                    _ap(ee[:, 0, :], [[N, IL], [1, 16]], extra_off=16))
                nc.vector.tensor_add(
                    _ap(tmp, [[16, IL], [1, 8]]),
                    _ap(tmp, [[16, IL], [1, 8]]),
                    _ap(tmp, [[16, IL], [1, 8]], extra_off=8))
                nc.vector.tensor_add(
                    _ap(tmp, [[16, IL], [1, 4]]),
                    _ap(tmp, [[16, IL], [1, 4]]),
                    _ap(tmp, [[16, IL], [1, 4]], extra_off=4))
                nc.vector.tensor_add(
                    _ap(tmp, [[16, IL], [1, 2]]),
                    _ap(tmp, [[16, IL], [1, 2]]),
                    _ap(tmp, [[16, IL], [1, 2]], extra_off=2))
                nc.vector.tensor_add(
                    _ap(den, [[1, IL]]),
                    _ap(tmp, [[16, IL], [1, 1]]),
                    _ap(tmp, [[16, IL], [1, 1]], extra_off=1))
                nc.vector.reciprocal(out=den[:], in_=den[:])
                nc.vector.tensor_mul(
                    ee[:], ee[:], _ap(den, [[1, IL], [0, N]]))
                # ---- wsum ----
                for blk in range(NBLK):
                    i0 = blk * BI
                    nc.vector.tensor_mul(
                        tmp4[:, :, :, :],
                        _ap(hat[:, i0, :], [[ND, BI], [N, D], [1, N]]),
                        _ap(ee[:, i0, :], [[N, BI], [0, D], [1, N]]))
                    nc.vector.tensor_add(
                        _ap(tmp, [[ND, 8], [1, ND]]),
                        _ap(tmp, [[ND, 8], [1, ND]]),
                        _ap(tmp, [[ND, 8], [1, ND]], extra_off=8 * ND))
                    nc.vector.tensor_add(
                        _ap(tmp, [[ND, 4], [1, ND]]),
                        _ap(tmp, [[ND, 4], [1, ND]]),
                        _ap(tmp, [[ND, 4], [1, ND]], extra_off=4 * ND))
                    nc.vector.tensor_add(
                        _ap(tmp, [[ND, 2], [1, ND]]),
                        _ap(tmp, [[ND, 2], [1, ND]]),
                        _ap(tmp, [[ND, 2], [1, ND]], extra_off=2 * ND))
                    nc.vector.tensor_add(
                        sblk[:, blk, :],
                        _ap(tmp, [[1, ND]]),
                        _ap(tmp, [[1, ND]], extra_off=ND))
                # cross-block tree: 9 -> 4 (+8 leftover) -> 2 -> 1
                nc.vector.tensor_add(
                    _ap(sblk, [[ND, 4], [1, ND]]),
                    _ap(sblk, [[ND, 4], [1, ND]]),
                    _ap(sblk, [[ND, 4], [1, ND]], extra_off=4 * ND))
                nc.vector.tensor_add(
                    _ap(sblk, [[ND, 2], [1, ND]]),
                    _ap(sblk, [[ND, 2], [1, ND]]),
                    _ap(sblk, [[ND, 2], [1, ND]], extra_off=2 * ND))
                nc.vector.tensor_add(
                    _ap(sblk, [[1, ND]]),
                    _ap(sblk, [[1, ND]]),
                    _ap(sblk, [[1, ND]], extra_off=ND))
                nc.vector.tensor_add(
                    _ap(sblk, [[1, ND]]),
                    _ap(sblk, [[1, ND]]),
                    _ap(sblk, [[1, ND]], extra_off=8 * ND))
                if r < ROUTINGS - 1:
                    allreduce_s()
                    squash(s_bf)
                else:
                    nc.vector.tensor_copy(s_sb, _ap(sblk, [[1, ND]]))
                    nc.sync.dma_start(out=out_d[:], in_=s_sb)

    nc.compile()
    return nc


_NC_CACHE = None


def kernel(inputs: np.ndarray, W: np.ndarray) -> np.ndarray:
    global _NC_CACHE
    if _NC_CACHE is None:
        _NC_CACHE = build_nc()
    nc = _NC_CACHE

    inputs = np.ascontiguousarray(inputs, dtype=np.float32)
    W = np.ascontiguousarray(W, dtype=np.float32)
    x_bf = inputs.astype(BF)                       # [B, I, k]
    w_kidn = W.transpose(3, 1, 2, 0).astype(BF)    # [k, i, d, n]

    xdf = np.ascontiguousarray(
        x_bf.transpose(1, 2, 0).reshape(KF, 16 * DIN, B)
        .transpose(1, 0, 2))                       # [(i16 k), c, b]
    wdf = np.ascontiguousarray(
        w_kidn.reshape(DIN, KF, 16, D * N)
        .transpose(2, 0, 1, 3)
        .reshape(16 * DIN, KF, D * N))             # [(i16 k), c, (d n)]

    in_maps = []
    for cix in range(NCORES):
        sl = slice(cix * IL, (cix + 1) * IL)
        xs = x_bf[:, sl, :]
        wsl = w_kidn[:, sl, :, :]
        xq = np.ascontiguousarray(
            xs.transpose(2, 1, 0).reshape(DIN, G, 4, B)
            .transpose(2, 0, 1, 3))                # [4, k, g, b]
        wq = np.ascontiguousarray(
            wsl.reshape(DIN, G, 4, D * N)
            .transpose(2, 0, 1, 3))                # [4, k, g, (d n)]
        in_maps.append({"xq": xq, "wq": wq, "xdf": xdf, "wdf": wdf})

    trace = bool(int(os.environ.get("CAPS_TRACE", "0")))
    res = bass_utils.run_bass_kernel_spmd(
        nc, in_maps, core_ids=list(range(NCORES)), trace=trace)
    if trace and res.exec_time_ns is not None:
        print(f"HW exec time: {res.exec_time_ns} ns")

    s = np.zeros((B, ND), dtype=np.float64)
    for cix in range(NCORES):
        s += res.results[cix]["out"].astype(np.float64)
    s = s.reshape(B, D, N).transpose(0, 2, 1)      # [b, n, d]
    s2 = np.sum(s * s, axis=-1, keepdims=True)
    scale = s2 / (1.0 + s2) / np.sqrt(s2 + EPS)
    return (scale * s).astype(np.float32)


# revision 11
# speedup vs baseline: 1.0862x; 1.0862x over previous
"""CapsuleLayer dynamic-routing kernel for 8 Trainium2 NeuronCores — v4.1.

I-sharding: each core owns 144 of the 1152 input capsules.

s0 = (1/N) sum_i hat_i over ALL 1152 i is computed LOCALLY on every core
from bf16 copies of the full X/W via a dense (i,k)-packed matmul chain
(72 K=128 steps) — no AllReduce #1.  A dummy collective (alone on the
gpsimd queue) at t=0 absorbs the ~50us first-collective bootstrap so
AllReduce #2 runs lean.

Create (PE, bf16): per-i hat matmuls 4x row-tiled (tile_position=(32q,0)),
4 i's -> one [128,2048] 4-bank PSUM tile; evac fp32->bf16 on ScalarE only,
so the DVE round-1 proj pass pipelines underneath.  hat is SBUF bf16
[b, i, (d,n)], n contiguous.

Routing (DVE, BI=16 blocks, hot TT ops in 2x bf16 mode):
  proj: tmp = hat_blk * rep4; log-tree over d -> bb[b,i,n]; per-block exp
  softmax: DVE tree over n + reciprocal
  wsum: tmp = hat_blk * c (bcast over d); log-tree over i -> sblk; cross tree
Round 2 ships per-core partial s2; host sums partials + squashes in numpy.
"""

import os
import numpy as np
import ml_dtypes

import concourse.bass as bass
import concourse.bacc as bacc
import concourse.tile as tile
import concourse.mybir as mybir
from concourse import bass_utils

B, I, DIN = 128, 1152, 8
N, D = 32, 16
ND = N * D  # 512
NCORES = 8
IL = I // NCORES          # 144
G = IL // 4               # 36 groups of 4 i's
WCH = 4                   # create stream chunk: 4 groups
NCH = G // WCH            # 9 waves
KF = I * DIN // 128       # 72 dense-packed K chunks for s0 (full I)
WV = 8                    # s0 W stream: 8 chunks per wave
NWV = KF // WV            # 9 waves
BI = 16                   # i-block for routing passes
NBLK = IL // BI           # 9
EPS = 1e-7
ROUTINGS = 3
F32 = mybir.dt.float32
BF16 = mybir.dt.bfloat16
BF = ml_dtypes.bfloat16


def _ap(ap: bass.AP, dims, extra_off=0) -> bass.AP:
    return bass.AP(tensor=ap.tensor, offset=ap.offset + extra_off,
                   ap=[ap.ap[0]] + list(dims))


def build_nc():
    nc = bacc.Bacc(
        "TRN2",
        target_bir_lowering=False,
        debug=False,
        enable_asserts=True,
        num_devices=NCORES,
    )
    xq_d = nc.dram_tensor("xq", [4, DIN, G, B], BF16, kind="ExternalInput").ap()
    wq_d = nc.dram_tensor("wq", [4, DIN, G, ND], BF16, kind="ExternalInput").ap()
    xdf_d = nc.dram_tensor("xdf", [128, KF, B], BF16, kind="ExternalInput").ap()
    wdf_d = nc.dram_tensor("wdf", [128, KF, ND], BF16, kind="ExternalInput").ap()
    out_d = nc.dram_tensor("out", [B, ND], F32, kind="ExternalOutput").ap()

    with tile.TileContext(nc) as tc:
        with (
            tc.tile_pool(name="big", bufs=1) as big,
            tc.tile_pool(name="ps", bufs=2, space="PSUM") as pspool,
            tc.tile_pool(name="dram", bufs=1, space="DRAM") as dram,
        ):
            hat = big.tile([B, IL, ND], BF16)           # 144 KB/part
            scratch = big.tile([B, 19456], BF16)        # 38 KB/part union
            bb = big.tile([B, IL, N], BF16)             # 9 KB
            ee = big.tile([B, IL, N], BF16)             # 9 KB
            big3 = big.tile([B, 3, ND], F32)            # 6 KB
            s_sb, outv, tsq = (big3[:, j, :] for j in range(3))
            smalls = big.tile([B, 8, N], F32)           # 1 KB
            s2, a1, r1, rt = (smalls[:, j, :] for j in range(4))
            eps_t = smalls[:, 4, 0:1]
            dum_src = smalls[:, 6, 0:8]
            dum_dst = smalls[:, 7, 0:8]
            den = big.tile([B, IL], F32)                # 0.6 KB

            def sv(off, dims):
                return _ap(scratch, dims, extra_off=off)

            # -- elem map (bf16) --
            # routing: tmp 0..8192 | rep4 8192..10240 | sblk(8) 10240..14336
            # create : xq waves 14336..15360 | wq waves 15360..19456
            # s0     : xdf 0..9216 | wdf waves 9216..13312, 13312..17408
            tmp = sv(0, [[ND, BI], [1, ND]])
            tmp4 = sv(0, [[ND, BI], [N, D], [1, N]])
            rep4 = sv(8192, [[ND, 4], [1, ND]])
            sblk = sv(10240, [[ND, 8], [1, ND]])
            xqw = [sv(14336 + j * 512, [[B, WCH], [1, B]]) for j in range(2)]
            wqw = [sv(15360 + j * 2048, [[ND, WCH], [1, ND]]) for j in range(2)]
            xdf_sb = sv(0, [[B, KF], [1, B]])
            wdfw = [sv(9216 + j * WV * ND, [[ND, WV], [1, ND]]) for j in range(2)]

            nc.vector.memset(eps_t, EPS)
            nc.vector.memset(dum_src, 0.0)

            # ---------- dummy collective: absorb CC bootstrap ----------
            dum_in = dram.tile([B, 8], F32, tag="dumin")
            dum_out = dram.tile([B, 8], F32, tag="dumout")
            nc.gpsimd.dma_start(out=dum_in[:], in_=dum_src)
            nc.gpsimd.collective_compute(
                "AllReduce", mybir.AluOpType.add,
                replica_groups=[list(range(NCORES))],
                ins=[dum_in.opt()], outs=[dum_out.opt()])
            nc.gpsimd.dma_start(out=dum_dst, in_=dum_out[:])

            # ---------- s0 over ALL i (bf16 dense chain, no collective) ---
            # xdf + alternate wdf waves split over sync/tensor DMA queues
            nc.sync.dma_start(out=xdf_sb, in_=xdf_d[:, :, :])
            s0ps = pspool.tile([B, ND], F32, tag="ps")
            for wv in range(NWV):
                wt = wdfw[wv % 2]
                eng = nc.sync if wv % 2 == 0 else nc.scalar
                eng.dma_start(
                    out=wt, in_=wdf_d[:, wv * WV:(wv + 1) * WV, :])
                for j in range(WV):
                    c = wv * WV + j
                    nc.tensor.matmul(
                        s0ps[:], lhsT=xdf_sb[:, c, :], rhs=wt[:, j, :],
                        start=(c == 0), stop=(c == KF - 1),
                    )
            nc.scalar.mul(out=s_sb, in_=s0ps[:], mul=1.0 / N)

            s_bf = sblk[:, 1, :]  # bf16 AllReduce result staging

            def allreduce_s():
                # bf16 payload halves the ring traffic; input is the
                # already-bf16 cross-tree result in sblk[:,0,:]
                ar_in = dram.tile([B, ND], BF16, tag="arin")
                ar_out = dram.tile([B, ND], BF16, tag="arout")
                nc.gpsimd.dma_start(out=ar_in[:], in_=sblk[:, 0, :])
                nc.gpsimd.collective_compute(
                    "AllReduce", mybir.AluOpType.add,
                    replica_groups=[list(range(NCORES))],
                    ins=[ar_in.opt()], outs=[ar_out.opt()])
                nc.gpsimd.dma_start(out=s_bf, in_=ar_out[:])

            # ---------- create hat (evac on ScalarE only) ----------
            for ch in range(NCH):
                x_t, w_t = xqw[ch % 2], wqw[ch % 2]
                for q in range(4):
                    nc.sync.dma_start(
                        out=x_t[32 * q:32 * q + DIN, :, :],
                        in_=xq_d[q, :, ch * WCH:(ch + 1) * WCH, :])
                    nc.sync.dma_start(
                        out=w_t[32 * q:32 * q + DIN, :, :],
                        in_=wq_d[q, :, ch * WCH:(ch + 1) * WCH, :])
                for j in range(WCH):
                    g = ch * WCH + j
                    ps = pspool.tile([B, 4 * ND], F32, tag="ps")
                    for q in range(4):
                        nc.tensor.matmul(
                            ps[:, q * ND:(q + 1) * ND],
                            lhsT=x_t[32 * q:32 * q + DIN, j, :],
                            rhs=w_t[32 * q:32 * q + DIN, j, :],
                            start=True, stop=True,
                            tile_position=(32 * q, 0),
                        )
                    nc.scalar.copy(
                        out=_ap(hat[:, 4 * g, :], [[1, 4 * ND]]), in_=ps[:])

            def squash(src):
                nc.vector.tensor_mul(tsq, src, src)
                nc.vector.tensor_add(
                    _ap(tsq, [[N, 8], [1, N]]),
                    _ap(tsq, [[N, 8], [1, N]]),
                    _ap(tsq, [[N, 8], [1, N]], extra_off=8 * N))
                nc.vector.tensor_add(
                    _ap(tsq, [[N, 4], [1, N]]),
                    _ap(tsq, [[N, 4], [1, N]]),
                    _ap(tsq, [[N, 4], [1, N]], extra_off=4 * N))
                nc.vector.tensor_add(
                    _ap(tsq, [[N, 2], [1, N]]),
                    _ap(tsq, [[N, 2], [1, N]]),
                    _ap(tsq, [[N, 2], [1, N]], extra_off=2 * N))
                nc.vector.tensor_add(
                    s2, _ap(tsq, [[1, N]]), _ap(tsq, [[1, N]], extra_off=N))
                nc.scalar.add(out=a1, in_=s2, add=1.0)
                nc.vector.reciprocal(out=r1, in_=a1)
                nc.vector.tensor_mul(r1, r1, s2)
                nc.scalar.activation(
                    out=rt, in_=s2,
                    func=mybir.ActivationFunctionType.Sqrt,
                    bias=eps_t, scale=1.0)
                nc.vector.reciprocal(out=rt, in_=rt)
                nc.vector.tensor_mul(r1, r1, rt)
                nc.vector.tensor_mul(
                    _ap(outv, [[N, D], [1, N]]),
                    _ap(src, [[N, D], [1, N]]),
                    _ap(r1, [[0, D], [1, N]]))

            def fill_rep4():
                nc.vector.tensor_copy(rep4[:, 0, :], outv)
                nc.vector.tensor_copy(
                    _ap(rep4, [[1, ND]], extra_off=ND), _ap(rep4, [[1, ND]]))
                nc.vector.tensor_copy(
                    _ap(rep4, [[1, 2 * ND]], extra_off=2 * ND),
                    _ap(rep4, [[1, 2 * ND]]))

            # ---------- routing ----------
            squash(s_sb)

            for r in range(1, ROUTINGS):
                fill_rep4()
                for blk in range(NBLK):
                    i0 = blk * BI
                    nc.vector.tensor_mul(
                        tmp[:, :, :],
                        _ap(hat[:, i0, :], [[1, BI * ND]]),
                        _ap(rep4, [[0, 4], [1, 4 * ND]]))
                    nc.vector.tensor_add(
                        _ap(tmp4, [[ND, BI], [N, 8], [1, N]]),
                        _ap(tmp4, [[ND, BI], [N, 8], [1, N]]),
                        _ap(tmp4, [[ND, BI], [N, 8], [1, N]], extra_off=8 * N))
                    nc.vector.tensor_add(
                        _ap(tmp4, [[ND, BI], [N, 4], [1, N]]),
                        _ap(tmp4, [[ND, BI], [N, 4], [1, N]]),
                        _ap(tmp4, [[ND, BI], [N, 4], [1, N]], extra_off=4 * N))
                    nc.vector.tensor_add(
                        _ap(tmp4, [[ND, BI], [N, 2], [1, N]]),
                        _ap(tmp4, [[ND, BI], [N, 2], [1, N]]),
                        _ap(tmp4, [[ND, BI], [N, 2], [1, N]], extra_off=2 * N))
                    bb_blk = _ap(bb[:, i0, :], [[N, BI], [1, N]])
                    if r == 1:
                        nc.vector.tensor_add(
                            bb_blk,
                            _ap(tmp4, [[ND, BI], [1, N]]),
                            _ap(tmp4, [[ND, BI], [1, N]], extra_off=N))
                    else:
                        nc.vector.tensor_add(
                            _ap(tmp4, [[ND, BI], [1, N]]),
                            _ap(tmp4, [[ND, BI], [1, N]]),
                            _ap(tmp4, [[ND, BI], [1, N]], extra_off=N))
                        nc.vector.tensor_add(
                            bb_blk, bb_blk, _ap(tmp4, [[ND, BI], [1, N]]))
                    nc.scalar.activation(
                        out=_ap(ee[:, i0, :], [[1, BI * N]]),
                        in_=_ap(bb[:, i0, :], [[1, BI * N]]),
                        func=mybir.ActivationFunctionType.Exp,
                        bias=eps_t, scale=1.0)
                # ---- softmax denominator ----
                nc.vector.tensor_add(
                    _ap(tmp, [[16, IL], [1, 16]]),
                    _ap(ee[:, 0, :], [[N, IL], [1, 16]]),
                    _ap(ee[:, 0, :], [[N, IL], [1, 16]], extra_off=16))
                nc.vector.tensor_add(
                    _ap(tmp, [[16, IL], [1, 8]]),
                    _ap(tmp, [[16, IL], [1, 8]]),
                    _ap(tmp, [[16, IL], [1, 8]], extra_off=8))
                nc.vector.tensor_add(
                    _ap(tmp, [[16, IL], [1, 4]]),
                    _ap(tmp, [[16, IL], [1, 4]]),
                    _ap(tmp, [[16, IL], [1, 4]], extra_off=4))
                nc.vector.tensor_add(
                    _ap(tmp, [[16, IL], [1, 2]]),
                    _ap(tmp, [[16, IL], [1, 2]]),
                    _ap(tmp, [[16, IL], [1, 2]], extra_off=2))
                nc.vector.tensor_add(
                    _ap(den, [[1, IL]]),
                    _ap(tmp, [[16, IL], [1, 1]]),
                    _ap(tmp, [[16, IL], [1, 1]], extra_off=1))
                nc.vector.reciprocal(out=den[:], in_=den[:])
                nc.vector.tensor_mul(
                    ee[:], ee[:], _ap(den, [[1, IL], [0, N]]))
                # ---- wsum ----
                for blk in range(NBLK):
                    i0 = blk * BI
                    nc.vector.tensor_mul(
                        tmp4[:, :, :, :],
                        _ap(hat[:, i0, :], [[ND, BI], [N, D], [1, N]]),
                        _ap(ee[:, i0, :], [[N, BI], [0, D], [1, N]]))
                    nc.vector.tensor_add(
                        _ap(tmp, [[ND, 8], [1, ND]]),
                        _ap(tmp, [[ND, 8], [1, ND]]),
                        _ap(tmp, [[ND, 8], [1, ND]], extra_off=8 * ND))
                    nc.vector.tensor_add(
                        _ap(tmp, [[ND, 4], [1, ND]]),
                        _ap(tmp, [[ND, 4], [1, ND]]),
                        _ap(tmp, [[ND, 4], [1, ND]], extra_off=4 * ND))
                    nc.vector.tensor_add(
                        _ap(tmp, [[ND, 2], [1, ND]]),
                        _ap(tmp, [[ND, 2], [1, ND]]),
                        _ap(tmp, [[ND, 2], [1, ND]], extra_off=2 * ND))
                    if blk < 8:
                        nc.vector.tensor_add(
                            sblk[:, blk, :],
                            _ap(tmp, [[1, ND]]),
                            _ap(tmp, [[1, ND]], extra_off=ND))
                    else:
                        nc.vector.tensor_add(
                            _ap(tmp, [[1, ND]]),
                            _ap(tmp, [[1, ND]]),
                            _ap(tmp, [[1, ND]], extra_off=ND))
                        nc.vector.tensor_add(
                            sblk[:, 0, :], sblk[:, 0, :], _ap(tmp, [[1, ND]]))
                # cross-block tree: 8 -> 4 -> 2 -> 1
                nc.vector.tensor_add(
                    _ap(sblk, [[ND, 4], [1, ND]]),
                    _ap(sblk, [[ND, 4], [1, ND]]),
                    _ap(sblk, [[ND, 4], [1, ND]], extra_off=4 * ND))
                nc.vector.tensor_add(
                    _ap(sblk, [[ND, 2], [1, ND]]),
                    _ap(sblk, [[ND, 2], [1, ND]]),
                    _ap(sblk, [[ND, 2], [1, ND]], extra_off=2 * ND))
                nc.vector.tensor_add(
                    _ap(sblk, [[1, ND]]),
                    _ap(sblk, [[1, ND]]),
                    _ap(sblk, [[1, ND]], extra_off=ND))
                if r < ROUTINGS - 1:
                    allreduce_s()
                    squash(s_bf)
                else:
                    nc.vector.tensor_copy(s_sb, _ap(sblk, [[1, ND]]))
                    nc.sync.dma_start(out=out_d[:], in_=s_sb)

    nc.compile()
    return nc


_NC_CACHE = None


def kernel(inputs: np.ndarray, W: np.ndarray) -> np.ndarray:
    global _NC_CACHE
    if _NC_CACHE is None:
        _NC_CACHE = build_nc()
    nc = _NC_CACHE

    inputs = np.ascontiguousarray(inputs, dtype=np.float32)
    W = np.ascontiguousarray(W, dtype=np.float32)
    x_bf = inputs.astype(BF)                       # [B, I, k]
    w_kidn = W.transpose(3, 1, 2, 0).astype(BF)    # [k, i, d, n]

    xdf = np.ascontiguousarray(
        x_bf.transpose(1, 2, 0).reshape(KF, 16 * DIN, B)
        .transpose(1, 0, 2))                       # [(i16 k), c, b]
    wdf = np.ascontiguousarray(
        w_kidn.reshape(DIN, KF, 16, D * N)
        .transpose(2, 0, 1, 3)
        .reshape(16 * DIN, KF, D * N))             # [(i16 k), c, (d n)]

    in_maps = []
    for cix in range(NCORES):
        sl = slice(cix * IL, (cix + 1) * IL)
        xs = x_bf[:, sl, :]
        wsl = w_kidn[:, sl, :, :]
        xq = np.ascontiguousarray(
            xs.transpose(2, 1, 0).reshape(DIN, G, 4, B)
            .transpose(2, 0, 1, 3))                # [4, k, g, b]
        wq = np.ascontiguousarray(
            wsl.reshape(DIN, G, 4, D * N)
            .transpose(2, 0, 1, 3))                # [4, k, g, (d n)]
        in_maps.append({"xq": xq, "wq": wq, "xdf": xdf, "wdf": wdf})

    trace = bool(int(os.environ.get("CAPS_TRACE", "0")))
    res = bass_utils.run_bass_kernel_spmd(
        nc, in_maps, core_ids=list(range(NCORES)), trace=trace)
    if trace and res.exec_time_ns is not None:
        print(f"HW exec time: {res.exec_time_ns} ns")

    s = np.zeros((B, ND), dtype=np.float64)
    for cix in range(NCORES):
        s += res.results[cix]["out"].astype(np.float64)
    s = s.reshape(B, D, N).transpose(0, 2, 1)      # [b, n, d]
    s2 = np.sum(s * s, axis=-1, keepdims=True)
    scale = s2 / (1.0 + s2) / np.sqrt(s2 + EPS)
    return (scale * s).astype(np.float32)
